# revision 8
# baseline (speedup 1.0000x reference)
"""Trainium2 Bass kernel for the segmented block-diagonal linear layer.

out[b, (seg, v, i)] = sum_u x[b, (seg, u, i)] * W_seg[u, v] / sqrt(mu_seg)

Segments (mul_in, mul_out, ir_dim): (256,256,1) (128,128,3) (64,64,5) (32,32,7)
x: [100000, 1184] f32, weight: [1, 87040] f32 -> out: [100000, 1184] f32

Strategy: data-parallel over 8 NeuronCores (12500 rows each). Per core,
stream row tiles packed M_PACK consecutive batch rows per SBUF partition so
every DMA descriptor covers M_PACK contiguous DRAM rows (descriptor-overhead
amortization). Per row-slot: PE-transpose feature chunks so features land on
partitions, then matmul against host-prepared dense block-diagonal weight
chunks (the delta-interleave over the irrep dim is baked into zeros
host-side), which makes every matmul contiguous and makes outputs land
directly in the final feature order. Compute dtype is fp16 (cast during the
DMA load): the PE's fp32r mode is TF32-class (~10-bit mantissa) so fp16
matches its accuracy while streaming 2x faster and transposing in one pass
instead of two. HBM traffic stays fp32 on both sides.
"""

import sys

if "/opt/trn_rl_repo" not in sys.path:
    sys.path.insert(0, "/opt/trn_rl_repo")

import numpy as np

import concourse.bacc as bacc
import concourse.mybir as mybir
from concourse import masks, tile
from concourse.bass_utils import run_bass_kernel_spmd

SEGS = [(256, 256, 1), (128, 128, 3), (64, 64, 5), (32, 32, 7)]
IN_DIM = 1184
W_NUMEL = 87040
N_CORES = 8
M_PACK = 8  # batch rows packed per SBUF partition in the main tiles

# Transpose pieces: contiguous feature chunks of x (feat_lo, width).
# Piece p is staged at xT columns [128*p, 128*p + width), partitions [0, width).
PIECES = [
    (0, 128), (128, 128),                      # seg0 (256 feats)
    (256, 128), (384, 128), (512, 128),        # seg1 (384 feats)
    (640, 128), (768, 128), (896, 64),         # seg2 (320 feats)
    (960, 128), (1088, 96),                    # seg3 (224 feats)
]

# Segment matmul plan: (piece_indices, psum_bank, psum_col_lo, n_cols, out_feat_lo, out_width, copy_engine)
# seg3 shares bank b0 with seg0 (cols 256:512, column-padded to 256).
SEG_PLAN = [
    ([0, 1], "b0", 0, 256, 0, 256, "act"),
    ([2, 3, 4], "b1", 0, 384, 256, 384, "act"),
    ([5, 6, 7], "b2", 0, 320, 640, 320, "vec"),
    ([8, 9], "b0", 256, 256, 960, 224, "act"),
]

_BUILD_CACHE = {}


def _prepare_dense_weights(weight):
    """Host-side: expand the flat weight into dense per-segment block matrices
    D[u*d+i, v*d+j] = W[u,v] * (i==j) / sqrt(mu), split into <=128-row chunks
    (seg3 column-padded to 256), cast to fp16 for the PE."""
    w = np.asarray(weight, dtype=np.float32).reshape(-1)
    chunks = []
    off = 0
    for si, (mu, mv, d) in enumerate(SEGS):
        W = w[off : off + mu * mv].reshape(mu, mv) * np.float32(1.0 / np.sqrt(mu))
        off += mu * mv
        D = np.zeros((mu * d, mv * d), dtype=np.float32)
        for i in range(d):
            D[i::d, i::d] = W
        if si == 3:
            Dp = np.zeros((mu * d, 256), dtype=np.float32)
            Dp[:, : mv * d] = D
            D = Dp
        for r0 in range(0, D.shape[0], 128):
            chunks.append(np.ascontiguousarray(D[r0 : r0 + 128]).astype(np.float16))
    return chunks  # 10 chunks, aligned with PIECES order


def _build(rows_per_core, w_shapes):
    key = (rows_per_core, tuple(w_shapes))
    if key in _BUILD_CACHE:
        return _BUILD_CACHE[key]

    f32 = mybir.dt.float32
    f16 = mybir.dt.float16

    nc = bacc.Bacc("TRN2", target_bir_lowering=False, debug=False)
    x_d = nc.declare_dram_parameter("x", [rows_per_core, IN_DIM], f32, isOutput=False)
    w_d = [
        nc.declare_dram_parameter(f"wd{i}", list(s), f16, isOutput=False)
        for i, s in enumerate(w_shapes)
    ]
    y_d = nc.declare_dram_parameter("y", [rows_per_core, IN_DIM], f32, isOutput=True)

    # Main tiles: M_PACK*128 rows, partition p holds rows r0 + M_PACK*p ...
    # + M_PACK-1. Tail tiles: one row per partition.
    main_rows = M_PACK * 128
    n_main = rows_per_core // main_rows
    tail = rows_per_core - n_main * main_rows
    tail_tiles = []
    while tail > 0:
        t = min(tail, 128)
        tail_tiles.append(t)
        tail -= t

    with tile.TileContext(nc) as tc:
        with (
            tc.tile_pool(name="wpool", bufs=1) as wpool,
            tc.tile_pool(name="xpool", bufs=3) as xpool,
            tc.tile_pool(name="xtpool", bufs=3) as xtpool,
            tc.tile_pool(name="ypool", bufs=3) as ypool,
            tc.tile_pool(name="stagp", bufs=2, space="PSUM") as stagp,
            tc.tile_pool(name="outp", bufs=2, space="PSUM") as outp,
        ):
            ident = wpool.tile([128, 128], f16)
            masks.make_identity(nc, ident[:])
            wts = []
            for i, s in enumerate(w_shapes):
                wt = wpool.tile(list(s), f16, name=f"wsb{i}")
                nc.sync.dma_start(out=wt[:], in_=w_d[i][:, :])
                wts.append(wt)

            GROUPS = [0, 4, 8]  # transpose-piece group starts

            def emit_tgroup(slot, g0):
                """Transpose one group of pieces into PSUM staging, then
                DVE-copy into the slot's xT tile."""
                xt, _, xT, _, j, rows = slot
                group = PIECES[g0 : g0 + 4]
                stag = stagp.tile([128, 512], f16, name="stag")
                for k, (flo, width) in enumerate(group):
                    nc.tensor.transpose(
                        stag[:width, k * 128 : k * 128 + rows],
                        xt[:rows, j * IN_DIM + flo : j * IN_DIM + flo + width],
                        ident[:rows, :rows],
                    )
                ncols = len(group) * 128
                nc.vector.tensor_copy(
                    xT[:, g0 * 128 : g0 * 128 + ncols], stag[:, :ncols]
                )

            def emit_seg_mms(slot, seg_idx):
                _, _, xT, pb, _, rows = slot
                pcs, bank, clo, n, _flo, _fw, _eng = SEG_PLAN[seg_idx]
                for jj, p in enumerate(pcs):
                    width = PIECES[p][1]
                    nc.tensor.matmul(
                        pb[bank][:rows, clo : clo + n],
                        xT[:width, p * 128 : p * 128 + rows],
                        wts[p][:width, :n],
                        start=(jj == 0),
                        stop=(jj == len(pcs) - 1),
                    )

            def emit_copies(slot):
                _, yt, _, pb, j, rows = slot
                for _pcs, bank, clo, _n, flo, fw, eng in SEG_PLAN:
                    src = pb[bank][:rows, clo : clo + fw]
                    dst = yt[:rows, j * IN_DIM + flo : j * IN_DIM + flo + fw]
                    if eng == "act":
                        nc.scalar.copy(out=dst, in_=src)
                    else:
                        nc.vector.tensor_copy(dst, src)

            # Software-pipeline the row-slots: interleave slot s+1's
            # transposes with slot s's matmuls so the PE never sees a long
            # matmul-free window (HAM would re-throttle the clock after
            # ~3.4us without matmul activity).
            pending = None  # slot whose matmuls have not been emitted yet
            out_dmas = []  # deferred output DMA emissions

            def start_slot(xt, yt, j, rows, finishes_tile):
                nonlocal pending
                xT = xtpool.tile([128, 128 * len(PIECES)], f16, name="xT")
                pb = {
                    "b0": outp.tile([128, 512], f32, name="pb0"),
                    "b1": outp.tile([128, 384], f32, name="pb1"),
                    "b2": outp.tile([128, 320], f32, name="pb2"),
                }
                slot = (xt, yt, xT, pb, j, rows)
                if pending is None:
                    for g in GROUPS:
                        emit_tgroup(slot, g)
                else:
                    prev = pending[0]
                    emit_tgroup(slot, GROUPS[0])
                    emit_seg_mms(prev, 0)
                    emit_tgroup(slot, GROUPS[1])
                    emit_seg_mms(prev, 1)
                    emit_tgroup(slot, GROUPS[2])
                    emit_seg_mms(prev, 2)
                    emit_seg_mms(prev, 3)
                    finish_pending()
                pending = (slot, finishes_tile)

            def finish_pending(emit_mms=False):
                nonlocal pending
                if pending is None:
                    return
                slot, fin = pending
                if emit_mms:
                    for si in range(4):
                        emit_seg_mms(slot, si)
                emit_copies(slot)
                if fin is not None:
                    fin()
                pending = None

            r0 = 0
            for _ in range(n_main):
                xt = xpool.tile([128, M_PACK * IN_DIM], f16, name="xt")
                src = x_d[r0 : r0 + main_rows, :].rearrange(
                    "(p m) f -> p (m f)", m=M_PACK
                )
                nc.gpsimd.dma_start(out=xt[:], in_=src)
                yt = ypool.tile([128, M_PACK * IN_DIM], f32, name="yt")

                def fin(yt=yt, r0=r0):
                    dst = y_d[r0 : r0 + main_rows, :].rearrange(
                        "(p m) f -> p (m f)", m=M_PACK
                    )
                    nc.sync.dma_start(out=dst, in_=yt[:])

                for j in range(M_PACK):
                    start_slot(xt, yt, j, 128, fin if j == M_PACK - 1 else None)
                r0 += main_rows

            for rows in tail_tiles:
                xt = xpool.tile([128, M_PACK * IN_DIM], f16, name="xt")
                nc.gpsimd.dma_start(
                    out=xt[:rows, :IN_DIM], in_=x_d[r0 : r0 + rows, :]
                )
                yt = ypool.tile([128, M_PACK * IN_DIM], f32, name="yt")

                def fin(yt=yt, r0=r0, rows=rows):
                    nc.sync.dma_start(
                        out=y_d[r0 : r0 + rows, :], in_=yt[:rows, :IN_DIM]
                    )

                start_slot(xt, yt, 0, rows, fin)
                r0 += rows

            finish_pending(emit_mms=True)

    nc.compile()
    _BUILD_CACHE[key] = nc
    return nc


def _run(x, weight, trace=False, trace_kwargs=None):
    x = np.ascontiguousarray(np.asarray(x, dtype=np.float32))
    batch = x.shape[0]
    assert batch % N_CORES == 0, f"batch {batch} not divisible by {N_CORES}"
    rows_per_core = batch // N_CORES

    wchunks = _prepare_dense_weights(weight)
    nc = _build(rows_per_core, [c.shape for c in wchunks])

    in_maps = []
    for c in range(N_CORES):
        m = {"x": x[c * rows_per_core : (c + 1) * rows_per_core]}
        for i, wc in enumerate(wchunks):
            m[f"wd{i}"] = wc
        in_maps.append(m)

    kwargs = {}
    if trace:
        kwargs["trace"] = True
        if trace_kwargs:
            kwargs["trace_kwargs"] = trace_kwargs
    res = run_bass_kernel_spmd(nc, in_maps, list(range(N_CORES)), **kwargs)
    out = np.concatenate([res.results[c]["y"] for c in range(N_CORES)], axis=0)
    return out.astype(np.float32, copy=False), res


def kernel(x, weight):
    out, _ = _run(x, weight)
    return out


# revision 9
# speedup vs baseline: 1.2265x; 1.2265x over previous
"""Trainium2 Bass kernel for the segmented block-diagonal linear layer.

out[b, (seg, v, i)] = sum_u x[b, (seg, u, i)] * W_seg[u, v] / sqrt(mu_seg)

Segments (mul_in, mul_out, ir_dim): (256,256,1) (128,128,3) (64,64,5) (32,32,7)
x: [100000, 1184] f32, weight: [1, 87040] f32 -> out: [100000, 1184] f32

Strategy: data-parallel over 8 NeuronCores (12500 rows each). Per core,
stream row tiles packed M_PACK consecutive batch rows per SBUF partition so
every DMA descriptor covers M_PACK contiguous DRAM rows (descriptor-overhead
amortization). Per row-slot: PE-transpose feature chunks so features land on
partitions, then matmul against host-prepared dense block-diagonal weight
chunks (the delta-interleave over the irrep dim is baked into zeros
host-side), which makes every matmul contiguous and makes outputs land
directly in the final feature order. Compute dtype is fp16 (cast during the
DMA load): the PE's fp32r mode is TF32-class (~10-bit mantissa) so fp16
matches its accuracy while streaming 2x faster and transposing in one pass
instead of two. HBM traffic stays fp32 on both sides.
"""

import sys

if "/opt/trn_rl_repo" not in sys.path:
    sys.path.insert(0, "/opt/trn_rl_repo")

import numpy as np

import concourse.bacc as bacc
import concourse.mybir as mybir
from concourse import masks, tile
from concourse.bass_utils import run_bass_kernel_spmd

SEGS = [(256, 256, 1), (128, 128, 3), (64, 64, 5), (32, 32, 7)]
IN_DIM = 1184
W_NUMEL = 87040
N_CORES = 8
M_PACK = 4  # batch rows packed per SBUF partition in the main tiles

# Transpose pieces: contiguous feature chunks of x (feat_lo, width).
# Piece p is staged at xT columns [128*p, 128*p + width), partitions [0, width).
PIECES = [
    (0, 128), (128, 128),                      # seg0 (256 feats)
    (256, 128), (384, 128), (512, 128),        # seg1 (384 feats)
    (640, 128), (768, 128), (896, 64),         # seg2 (320 feats)
    (960, 128), (1088, 96),                    # seg3 (224 feats)
]

# Segment matmul plan: (piece_indices, psum_bank, psum_col_lo, n_cols, out_feat_lo, out_width, copy_engine)
# seg3 shares bank b0 with seg0 (cols 256:512, column-padded to 256).
SEG_PLAN = [
    ([0, 1], "b0", 0, 256, 0, 256, "act"),
    ([2, 3, 4], "b1", 0, 384, 256, 384, "act"),
    ([5, 6, 7], "b2", 0, 320, 640, 320, "vec"),
    ([8, 9], "b0", 256, 256, 960, 224, "act"),
]

_BUILD_CACHE = {}


def _prepare_dense_weights(weight):
    """Host-side: expand the flat weight into dense per-segment block matrices
    D[u*d+i, v*d+j] = W[u,v] * (i==j) / sqrt(mu), split into <=128-row chunks
    (seg3 column-padded to 256), cast to fp16 for the PE."""
    w = np.asarray(weight, dtype=np.float32).reshape(-1)
    chunks = []
    off = 0
    for si, (mu, mv, d) in enumerate(SEGS):
        W = w[off : off + mu * mv].reshape(mu, mv) * np.float32(1.0 / np.sqrt(mu))
        off += mu * mv
        D = np.zeros((mu * d, mv * d), dtype=np.float32)
        for i in range(d):
            D[i::d, i::d] = W
        if si == 3:
            Dp = np.zeros((mu * d, 256), dtype=np.float32)
            Dp[:, : mv * d] = D
            D = Dp
        for r0 in range(0, D.shape[0], 128):
            chunks.append(np.ascontiguousarray(D[r0 : r0 + 128]).astype(np.float16))
    return chunks  # 10 chunks, aligned with PIECES order


def _build(rows_per_core, w_shapes):
    key = (rows_per_core, tuple(w_shapes))
    if key in _BUILD_CACHE:
        return _BUILD_CACHE[key]

    f32 = mybir.dt.float32
    f16 = mybir.dt.float16

    nc = bacc.Bacc("TRN2", target_bir_lowering=False, debug=False)
    x_d = nc.declare_dram_parameter("x", [rows_per_core, IN_DIM], f32, isOutput=False)
    w_d = [
        nc.declare_dram_parameter(f"wd{i}", list(s), f16, isOutput=False)
        for i, s in enumerate(w_shapes)
    ]
    y_d = nc.declare_dram_parameter("y", [rows_per_core, IN_DIM], f32, isOutput=True)

    # Main tiles: M_PACK*128 rows, partition p holds rows r0 + M_PACK*p ...
    # + M_PACK-1. Tail tiles: one row per partition.
    main_rows = M_PACK * 128
    n_main = rows_per_core // main_rows
    tail = rows_per_core - n_main * main_rows
    tail_tiles = []
    while tail > 0:
        t = min(tail, 128)
        tail_tiles.append(t)
        tail -= t

    with tile.TileContext(nc) as tc:
        with (
            tc.tile_pool(name="wpool", bufs=1) as wpool,
            tc.tile_pool(name="xpool", bufs=3) as xpool,
            tc.tile_pool(name="xtpool", bufs=3) as xtpool,
            tc.tile_pool(name="ypool", bufs=3) as ypool,
            tc.tile_pool(name="stagp", bufs=2, space="PSUM") as stagp,
            tc.tile_pool(name="outp", bufs=2, space="PSUM") as outp,
        ):
            ident = wpool.tile([128, 128], f16)
            masks.make_identity(nc, ident[:])
            wts = []
            for i, s in enumerate(w_shapes):
                wt = wpool.tile(list(s), f16, name=f"wsb{i}")
                nc.sync.dma_start(out=wt[:], in_=w_d[i][:, :])
                wts.append(wt)

            GROUPS = [0, 4, 8]  # transpose-piece group starts

            def emit_tgroup(slot, g0):
                """Transpose one group of pieces into PSUM staging, then
                DVE-copy into the slot's xT tile."""
                xt, _, xT, _, j, rows = slot
                group = PIECES[g0 : g0 + 4]
                stag = stagp.tile([128, 512], f16, name="stag")
                for k, (flo, width) in enumerate(group):
                    nc.tensor.transpose(
                        stag[:width, k * 128 : k * 128 + rows],
                        xt[:rows, j * IN_DIM + flo : j * IN_DIM + flo + width],
                        ident[:rows, :rows],
                    )
                ncols = len(group) * 128
                nc.vector.tensor_copy(
                    xT[:, g0 * 128 : g0 * 128 + ncols], stag[:, :ncols]
                )

            def emit_seg_mms(slot, seg_idx):
                _, _, xT, pb, _, rows = slot
                pcs, bank, clo, n, _flo, _fw, _eng = SEG_PLAN[seg_idx]
                for jj, p in enumerate(pcs):
                    width = PIECES[p][1]
                    nc.tensor.matmul(
                        pb[bank][:rows, clo : clo + n],
                        xT[:width, p * 128 : p * 128 + rows],
                        wts[p][:width, :n],
                        start=(jj == 0),
                        stop=(jj == len(pcs) - 1),
                    )

            def emit_copies(slot):
                _, yt, _, pb, j, rows = slot
                for _pcs, bank, clo, _n, flo, fw, eng in SEG_PLAN:
                    src = pb[bank][:rows, clo : clo + fw]
                    dst = yt[:rows, j * IN_DIM + flo : j * IN_DIM + flo + fw]
                    if eng == "act":
                        nc.scalar.copy(out=dst, in_=src)
                    else:
                        nc.vector.tensor_copy(dst, src)

            # Software-pipeline the row-slots: interleave slot s+1's
            # transposes with slot s's matmuls so the PE never sees a long
            # matmul-free window (HAM would re-throttle the clock after
            # ~3.4us without matmul activity).
            pending = None  # slot whose matmuls have not been emitted yet
            out_dmas = []  # deferred output DMA emissions

            def start_slot(xt, yt, j, rows, finishes_tile):
                nonlocal pending
                xT = xtpool.tile([128, 128 * len(PIECES)], f16, name="xT")
                pb = {
                    "b0": outp.tile([128, 512], f32, name="pb0"),
                    "b1": outp.tile([128, 384], f32, name="pb1"),
                    "b2": outp.tile([128, 320], f32, name="pb2"),
                }
                slot = (xt, yt, xT, pb, j, rows)
                if pending is None:
                    for g in GROUPS:
                        emit_tgroup(slot, g)
                else:
                    prev = pending[0]
                    emit_tgroup(slot, GROUPS[0])
                    emit_seg_mms(prev, 0)
                    emit_tgroup(slot, GROUPS[1])
                    emit_seg_mms(prev, 1)
                    emit_tgroup(slot, GROUPS[2])
                    emit_seg_mms(prev, 2)
                    emit_seg_mms(prev, 3)
                    finish_pending()
                pending = (slot, finishes_tile)

            def finish_pending(emit_mms=False):
                nonlocal pending
                if pending is None:
                    return
                slot, fin = pending
                if emit_mms:
                    for si in range(4):
                        emit_seg_mms(slot, si)
                emit_copies(slot)
                if fin is not None:
                    fin()
                pending = None

            r0 = 0
            for _ in range(n_main):
                xt = xpool.tile([128, M_PACK * IN_DIM], f16, name="xt")
                src = x_d[r0 : r0 + main_rows, :].rearrange(
                    "(p m) f -> p (m f)", m=M_PACK
                )
                nc.gpsimd.dma_start(out=xt[:], in_=src)
                yt = ypool.tile([128, M_PACK * IN_DIM], f32, name="yt")

                def fin(yt=yt, r0=r0):
                    dst = y_d[r0 : r0 + main_rows, :].rearrange(
                        "(p m) f -> p (m f)", m=M_PACK
                    )
                    nc.sync.dma_start(out=dst, in_=yt[:])

                for j in range(M_PACK):
                    start_slot(xt, yt, j, 128, fin if j == M_PACK - 1 else None)
                r0 += main_rows

            for rows in tail_tiles:
                xt = xpool.tile([128, M_PACK * IN_DIM], f16, name="xt")
                nc.gpsimd.dma_start(
                    out=xt[:rows, :IN_DIM], in_=x_d[r0 : r0 + rows, :]
                )
                yt = ypool.tile([128, M_PACK * IN_DIM], f32, name="yt")

                def fin(yt=yt, r0=r0, rows=rows):
                    nc.sync.dma_start(
                        out=y_d[r0 : r0 + rows, :], in_=yt[:rows, :IN_DIM]
                    )

                start_slot(xt, yt, 0, rows, fin)
                r0 += rows

            finish_pending(emit_mms=True)

    nc.compile()
    _BUILD_CACHE[key] = nc
    return nc


def _run(x, weight, trace=False, trace_kwargs=None):
    x = np.ascontiguousarray(np.asarray(x, dtype=np.float32))
    batch = x.shape[0]
    assert batch % N_CORES == 0, f"batch {batch} not divisible by {N_CORES}"
    rows_per_core = batch // N_CORES

    wchunks = _prepare_dense_weights(weight)
    nc = _build(rows_per_core, [c.shape for c in wchunks])

    in_maps = []
    for c in range(N_CORES):
        m = {"x": x[c * rows_per_core : (c + 1) * rows_per_core]}
        for i, wc in enumerate(wchunks):
            m[f"wd{i}"] = wc
        in_maps.append(m)

    kwargs = {}
    if trace:
        kwargs["trace"] = True
        if trace_kwargs:
            kwargs["trace_kwargs"] = trace_kwargs
    res = run_bass_kernel_spmd(nc, in_maps, list(range(N_CORES)), **kwargs)
    out = np.concatenate([res.results[c]["y"] for c in range(N_CORES)], axis=0)
    return out.astype(np.float32, copy=False), res


def kernel(x, weight):
    out, _ = _run(x, weight)
    return out
